# revision 14
# baseline (speedup 1.0000x reference)
"""Distributed Trainium2 Bass kernel for AlignmentContrastiveLoss.

Reference computation (B=256, L_im=37, L_s=33, D=1024):
    im  = l2norm(im_set)[:, 1:, :]   masked by im_len-1     [B, 36, D]
    s   = l2norm(s_seq)[:, 1:-2, :]  masked by s_len-3      [B, 30, D]
    align[b,c,i,j] = im[b,i] . s[c,j]   (masked entries -> 0)
    scores[b,c] = sum_j max_i align[b,c,i,j]
    loss = sum_b relu(M + max_{c!=b} scores[b,c] - scores[b,b])
         + sum_c relu(M + max_{b!=c} scores[b,c] - scores[c,c])

Sharding: image batch axis across 8 cores (32 images/core); every core
holds the full sentence set (replicated via its input map).  Each core
computes its 32x256 block of scores via fp32r matmuls (PE), max-over-i
on DVE directly from PSUM, the j-sum via small 0/1 "G" matmuls into two
per-core scoresT accumulators [256 x 32], then per-core partial stats
(col-max / diag / row-hinge) are AllGathered (768 floats) and every core
redundantly computes the final scalar.  s norms are computed sharded and
AllGathered (960 floats each) instead of redundantly per-core.
"""

import os
import sys

import numpy as np

for _p in ("/opt/trn_rl_repo", "/root/.axon_site/_ro/trn_rl_repo"):
    if os.path.isdir(_p) and _p not in sys.path:
        sys.path.append(_p)

import concourse.bass as bass
import concourse.mybir as mybir
import concourse.tile as tile
from concourse import bacc
from concourse.bass_utils import run_bass_kernel_spmd


def _ensure_axon_hooks():
    """Some agent images ship an ``antenv`` without ``axon_hooks``, but
    bass_utils hard-imports it when trace=True.  Provide the registry and,
    when libaxon_pjrt.so is available, the real NTFF profile hook."""
    import types

    try:
        import antenv.axon_hooks  # noqa: F401
        return
    except ImportError:
        pass
    try:
        import antenv
    except ImportError:
        return
    mod = types.ModuleType("antenv.axon_hooks")
    mod._hook = None
    mod.set_axon_ntff_profile_hook = lambda h: setattr(mod, "_hook", h)
    mod.get_axon_ntff_profile_hook = lambda: mod._hook
    sys.modules["antenv.axon_hooks"] = mod
    antenv.axon_hooks = mod
    so_path = "/opt/axon/libaxon_pjrt.so"
    try:
        import trn_agent_boot.trn_boot as _tb
        if os.path.exists(so_path):
            mod._hook = _tb._ntff_profile_via_ctypes(so_path)
    except Exception:
        pass


_ensure_axon_hooks()

F32 = mybir.dt.float32
F32R = mybir.dt.float32r
I32 = mybir.dt.int32
AX = mybir.AxisListType
ALU = mybir.AluOpType
ACT = mybir.ActivationFunctionType

NCORES = 8
B, LI, LS, D = 256, 36, 30, 1024
BL = B // NCORES            # 32 images / core
BI = BL * LI                # 1152 im rows / core
CJ = B * LS                 # 7680 (c,j) rows
NT = CJ // 128              # 60 M-tiles
NRT = BI // 128             # 9 im row-tiles
KC = D // 128               # 8 contraction chunks
SJ = CJ // NCORES           # 960 s rows / core (norm shard)
WROWS = 960                 # rows per 32-sentence window
NCHUNKS = [(0, 432, 12), (432, 432, 12), (864, 288, 8)]  # (off, width, n_images)
MARGIN, EPS, NEG = 0.2, 1e-12, -1.0e9

LAST_RESULT = None  # BassKernelResults of the most recent run (for test harness)


# ---------------------------------------------------------------------------
# compile-time tables
# ---------------------------------------------------------------------------

HALF_T = NT // 2  # 30 M-tiles per 128-sentence half


def _gmat_host():
    """G[p, 128t + cl] = 1 where row (128t+p) belongs to local sentence cl
    of tile t's half; G_t.T @ mx_t sums words j into scoresT[half] rows."""
    g = np.zeros((128, NT * 128), np.float32)
    for t in range(NT):
        h = t // HALF_T
        p = np.arange(128)
        cl = (128 * t + p) // LS - 128 * h
        g[p, 128 * t + cl] = 1.0
    return g


def _core_masks(m):
    pos0 = np.zeros((128, 32), np.float32)
    pos1 = np.zeros((128, 32), np.float32)
    tgt = pos0 if m < 4 else pos1
    b = np.arange(32)
    tgt[32 * (m % 4) + b, b] = 1.0
    return pos0, pos1, np.ascontiguousarray(pos0.T), np.ascontiguousarray(pos1.T)


# ---------------------------------------------------------------------------
# device program
# ---------------------------------------------------------------------------

def build_nc():
    nc = bacc.Bacc(None, target_bir_lowering=False, debug=False, num_devices=NCORES)

    imr_e = nc.declare_dram_parameter("imr", [BI, D], F32, isOutput=False)
    snr_e = nc.declare_dram_parameter("snr", [SJ, D], F32, isOutput=False)
    st_e = nc.declare_dram_parameter("st", [NT, 128, KC, 128], F32R, isOutput=False)
    imlen_e = nc.declare_dram_parameter("imlen", [BL], I32, isOutput=False)
    slen_e = nc.declare_dram_parameter("slen", [B], I32, isOutput=False)
    iota36_e = nc.declare_dram_parameter("iota36", [BL, LI], F32, isOutput=False)
    iota30_e = nc.declare_dram_parameter("iota30", [128, LS], F32, isOutput=False)
    ident_e = nc.declare_dram_parameter("ident", [128, 128], F32, isOutput=False)
    gmat_e = nc.declare_dram_parameter("gmat", [128, NT * 128], F32R, isOutput=False)
    pos0_e = nc.declare_dram_parameter("pos0", [128, 32], F32, isOutput=False)
    pos1_e = nc.declare_dram_parameter("pos1", [128, 32], F32, isOutput=False)
    post0_e = nc.declare_dram_parameter("post0", [32, 128], F32, isOutput=False)
    post1_e = nc.declare_dram_parameter("post1", [32, 128], F32, isOutput=False)
    out_e = nc.declare_dram_parameter("out", [1, 1], F32, isOutput=True)

    with tile.TileContext(nc) as tc:
        from contextlib import ExitStack

        with ExitStack() as ctx:
            dram = ctx.enter_context(tc.tile_pool(name="dram", bufs=1, space="DRAM"))
            const = ctx.enter_context(tc.tile_pool(name="const", bufs=1))
            small = ctx.enter_context(tc.tile_pool(name="small", bufs=1))
            stp = ctx.enter_context(tc.tile_pool(name="stp", bufs=3))
            mxp = ctx.enter_context(tc.tile_pool(name="mxp", bufs=3))
            pal = ctx.enter_context(tc.tile_pool(name="pal", bufs=3, space="PSUM"))
            psacc = ctx.enter_context(tc.tile_pool(name="psacc", bufs=1, space="PSUM"))

            # DRAM scratch
            imask_d = dram.tile([BI, 1], F32, tag="imask_d")
            smask_d = dram.tile([CJ, 1], F32, tag="smask_d")
            snorm_d = dram.tile([SJ, 1], F32, tag="snorm_d")
            snormall_d = dram.tile([CJ, 1], F32, tag="snormall_d")
            pay_d = dram.tile([768, 1], F32, tag="pay_d")
            ag2_d = dram.tile([NCORES * 768, 1], F32, tag="ag2_d")

            # ---- constants to SBUF ----
            ident = const.tile([128, 128], F32, tag="ident")
            nc.sync.dma_start(out=ident[:, :], in_=ident_e[:, :])
            gmat = const.tile([128, NT * 128], F32R, tag="gmat")
            nc.sync.dma_start(out=gmat[:, :], in_=gmat_e[:, :])
            iota36 = const.tile([BL, LI], F32, tag="iota36")
            nc.sync.dma_start(out=iota36[:, :], in_=iota36_e[:, :])
            iota30 = const.tile([128, LS], F32, tag="iota30")
            nc.sync.dma_start(out=iota30[:, :], in_=iota30_e[:, :])
            pos0 = const.tile([128, 32], F32, tag="pos0")
            nc.sync.dma_start(out=pos0[:, :], in_=pos0_e[:, :])
            pos1 = const.tile([128, 32], F32, tag="pos1")
            nc.sync.dma_start(out=pos1[:, :], in_=pos1_e[:, :])
            post0 = const.tile([32, 128], F32, tag="post0")
            nc.sync.dma_start(out=post0[:, :], in_=post0_e[:, :])
            post1 = const.tile([32, 128], F32, tag="post1")
            nc.sync.dma_start(out=post1[:, :], in_=post1_e[:, :])

            ones128 = const.tile([128, 1], F32, tag="ones128")
            nc.gpsimd.memset(ones128[:, :], 1.0)
            margin128 = const.tile([128, 1], F32, tag="margin128")
            nc.gpsimd.memset(margin128[:, :], MARGIN)

            # ---- masks from lengths ----
            imlen_i = small.tile([BL, 1], I32, tag="imlen_i")
            nc.sync.dma_start(out=imlen_i[:, :], in_=imlen_e[:])
            imlen_f = small.tile([BL, 1], F32, tag="imlen_f")
            nc.vector.tensor_copy(imlen_f[:, :], imlen_i[:, :])
            nc.vector.tensor_scalar_add(imlen_f[:, :], imlen_f[:, :], -1.0)
            mask36 = small.tile([BL, LI], F32, tag="mask36")
            nc.vector.tensor_scalar(
                out=mask36[:, :], in0=iota36[:, :], scalar1=imlen_f[:, :],
                scalar2=None, op0=ALU.is_lt,
            )
            nc.sync.dma_start(
                out=imask_d.rearrange("(b i) o -> b (i o)", b=BL),
                in_=mask36[:, :],
            )

            slen_i = small.tile([128, 2], I32, tag="slen_i")
            nc.sync.dma_start(
                out=slen_i[:, :],
                in_=slen_e.ap().rearrange("(h c) -> c h", h=2),
            )
            slen_f = small.tile([128, 2], F32, tag="slen_f")
            nc.vector.tensor_copy(slen_f[:, :], slen_i[:, :])
            nc.vector.tensor_scalar_add(slen_f[:, :], slen_f[:, :], -3.0)
            for h in range(2):
                mask30 = small.tile([128, LS], F32, tag="mask30")
                nc.vector.tensor_scalar(
                    out=mask30[:, :], in0=iota30[:, :], scalar1=slen_f[:, h:h + 1],
                    scalar2=None, op0=ALU.is_lt,
                )
                nc.sync.dma_start(
                    out=smask_d[3840 * h:3840 * (h + 1), :]
                    .rearrange("(c j) o -> c (j o)", c=128),
                    in_=mask30[:, :],
                )

            # ---- phase 1: sharded s sum-of-squares + AllGather ----
            prep = ctx.enter_context(tc.tile_pool(name="prep", bufs=2))
            for j in range(8):
                snr_t = prep.tile([120, D], F32, tag="ld")
                nc.sync.dma_start(out=snr_t[:, :], in_=snr_e[120 * j:120 * (j + 1), :])
                sq = prep.tile([120, D], F32, tag="sq")
                ssq = small.tile([120, 1], F32, tag=f"ssq{j}")
                nc.scalar.activation(sq[:, :], snr_t[:, :], ACT.Square,
                                     accum_out=ssq[:, :])
                nc.sync.dma_start(out=snorm_d[120 * j:120 * (j + 1), :], in_=ssq[:, :])
            nc.gpsimd.collective_compute(
                "AllGather", ALU.bypass,
                replica_groups=[list(range(NCORES))],
                ins=[snorm_d.opt()],
                outs=[snormall_d.opt()],
            )

            # ---- phase 2: im prep (norm+mask+scale, transpose to imt) ----
            imask_sb = small.tile([128, NRT], F32, tag="imask_sb")
            nc.sync.dma_start(
                out=imask_sb[:, :],
                in_=imask_d.rearrange("(rt p) o -> p (rt o)", p=128),
            )
            imt = const.tile([128, KC * BI], F32R, tag="imt")  # chunk k at cols [BI*k, BI*(k+1))
            ptp_ctx = tc.tile_pool(name="ptp", bufs=2, space="PSUM")
            ptp = ptp_ctx.__enter__()
            for rt in range(NRT):
                imr_t = prep.tile([128, D], F32, tag="ld")
                nc.sync.dma_start(out=imr_t[:, :], in_=imr_e[128 * rt:128 * (rt + 1), :])
                sq = prep.tile([128, D], F32, tag="sq")
                ssq = small.tile([128, 1], F32, tag="imssq")
                nc.scalar.activation(sq[:, :], imr_t[:, :], ACT.Square,
                                     accum_out=ssq[:, :])
                nrm = small.tile([128, 1], F32, tag="imnrm")
                nc.scalar.activation(nrm[:, :], ssq[:, :], ACT.Sqrt)
                nc.vector.tensor_scalar_max(nrm[:, :], nrm[:, :], EPS)
                rcp = small.tile([128, 1], F32, tag="imrcp")
                nc.vector.reciprocal(rcp[:, :], nrm[:, :])
                nc.vector.tensor_mul(rcp[:, :], rcp[:, :], imask_sb[:, rt:rt + 1])
                ims = prep.tile([128, D], F32, tag="ims")
                nc.scalar.mul(ims[:, :], imr_t[:, :], mul=rcp[:, :])
                for k in range(KC):
                    pst = ptp.tile([128, 128], F32, tag="pst")
                    nc.tensor.transpose(pst[:, :], ims[:, 128 * k:128 * (k + 1)],
                                        ident[:, :])
                    nc.scalar.copy(imt[:, BI * k + 128 * rt:BI * k + 128 * (rt + 1)],
                                   pst[:, :])

            ptp_ctx.__exit__(None, None, None)

            # ---- phase 3: s scale vector [128, NT] ----
            ssqall = small.tile([128, NT], F32, tag="ssqall")
            nc.sync.dma_start(
                out=ssqall[:, :],
                in_=snormall_d.rearrange("(t p) o -> p (t o)", p=128),
            )
            smask_sb = small.tile([128, NT], F32, tag="smask_sb")
            nc.sync.dma_start(
                out=smask_sb[:, :],
                in_=smask_d.rearrange("(t p) o -> p (t o)", p=128),
            )
            snrm = small.tile([128, NT], F32, tag="snrm")
            nc.scalar.activation(snrm[:, :], ssqall[:, :], ACT.Sqrt)
            nc.vector.tensor_scalar_max(snrm[:, :], snrm[:, :], EPS)
            sscale = small.tile([128, NT], F32, tag="sscale")
            nc.vector.reciprocal(sscale[:, :], snrm[:, :])
            nc.vector.tensor_mul(sscale[:, :], sscale[:, :], smask_sb[:, :])

            # ---- phase 4: main loop over 60 M-tiles ----
            s_ps = [psacc.tile([128, 32], F32, tag=f"S{h}", name=f"S{h}")
                    for h in range(2)]

            for t in range(NT):
                st_t = stp.tile([128, KC * 128], F32R, tag="st")
                nc.sync.dma_start(
                    out=st_t.rearrange("p (k c) -> p k c", k=KC),
                    in_=st_e[t, :, :, :],
                )
                mx = mxp.tile([128, 32], F32, tag="mx")
                ioff = 0
                for (noff, nw, nimg) in NCHUNKS:
                    ps = pal.tile([128, 432], F32, tag="al")
                    for k in range(KC):
                        nc.tensor.matmul(
                            ps[:, :nw],
                            lhsT=st_t[:, 128 * k:128 * (k + 1)],
                            rhs=imt[:, BI * k + noff:BI * k + noff + nw],
                            start=(k == 0), stop=(k == KC - 1),
                        )
                    nc.vector.tensor_reduce(
                        out=mx[:, ioff:ioff + nimg],
                        in_=ps.rearrange("p (g i) -> p g i", i=LI)[:, :nimg, :],
                        axis=AX.X, op=ALU.max,
                    )
                    ioff += nimg
                mx_r = mxp.tile([128, 32], F32R, tag="mx_r")
                nc.scalar.mul(mx_r[:, :], mx[:, :], mul=sscale[:, t:t + 1])
                nc.tensor.matmul(
                    s_ps[t // HALF_T][:, :],
                    lhsT=gmat[:, 128 * t:128 * (t + 1)],
                    rhs=mx_r[:, :],
                    start=(t % HALF_T == 0), stop=(t % HALF_T == HALF_T - 1),
                )

            # ---- phase 5: per-core stats + AllGather + final ----
            pepi = ctx.enter_context(tc.tile_pool(name="pepi", bufs=2, space="PSUM"))
            posm = [pos0, pos1]
            colpart = small.tile([128, 2], F32, tag="colpart")
            dcol = small.tile([128, 2], F32, tag="dcol")
            snd = [small.tile([128, 32], F32, tag=f"snd{h}", name=f"snd{h}")
                   for h in range(2)]
            trash = small.tile([128, 32], F32, tag="trash")
            for h in range(2):
                # diag extraction: accum_out = sum(S * pos)
                nc.vector.scalar_tensor_tensor(
                    out=trash[:, :], in0=s_ps[h][:, :], scalar=1.0, in1=posm[h][:, :],
                    op0=ALU.mult, op1=ALU.mult, accum_out=dcol[:, h:h + 1],
                )
                # mask own diag to -inf
                negm = small.tile([128, 32], F32, tag="negm")
                nc.vector.tensor_scalar_mul(negm[:, :], posm[h][:, :], NEG)
                nc.vector.tensor_add(snd[h][:, :], s_ps[h][:, :], negm[:, :])
                nc.vector.tensor_reduce(out=colpart[:, h:h + 1], in_=snd[h][:, :],
                                        axis=AX.X, op=ALU.max)
            # f32r twins of matmul operands (ACT rounds on write)
            posr = [small.tile([128, 32], F32R, tag=f"posr{h}", name=f"posr{h}")
                    for h in range(2)]
            nc.scalar.copy(posr[0][:, :], pos0[:, :])
            nc.scalar.copy(posr[1][:, :], pos1[:, :])
            postr = [small.tile([32, 128], F32R, tag=f"postr{h}", name=f"postr{h}")
                     for h in range(2)]
            nc.scalar.copy(postr[0][:, :], post0[:, :])
            nc.scalar.copy(postr[1][:, :], post1[:, :])
            dcolr = small.tile([128, 2], F32R, tag="dcolr")
            nc.scalar.copy(dcolr[:, :], dcol[:, :])
            onesr = const.tile([128, 1], F32R, tag="onesr")
            nc.scalar.copy(onesr[:, :], ones128[:, :])

            # row max via PE transpose of snd halves
            rm = small.tile([32, 2], F32, tag="rm")
            for h in range(2):
                stp_ps = pepi.tile([32, 128], F32, tag="epi")
                nc.tensor.transpose(stp_ps[:, :], snd[h][:, :], ident[:, :])
                nc.vector.tensor_reduce(out=rm[:, h:h + 1], in_=stp_ps[:, :],
                                        axis=AX.X, op=ALU.max)
            rowmax = small.tile([32, 1], F32, tag="rowmax")
            nc.vector.tensor_max(rowmax[:, :], rm[:, 0:1], rm[:, 1:2])
            # diag in row order: dfree = pos0.T @ dcol0 + pos1.T @ dcol1
            # fp32r matmuls need even moving-dim: run both halves at N=2;
            # exactly one of pos0/pos1 is nonzero per core so cross terms are 0
            dfree_ps = pepi.tile([32, 2], F32, tag="epi")
            nc.tensor.matmul(dfree_ps[:, :], lhsT=posr[0][:, :],
                             rhs=dcolr[:, :], start=True, stop=False)
            nc.tensor.matmul(dfree_ps[:, :], lhsT=posr[1][:, :],
                             rhs=dcolr[:, :], start=False, stop=True)
            dfree2 = small.tile([32, 2], F32, tag="dfree2")
            nc.scalar.copy(dfree2[:, :], dfree_ps[:, :])
            dfree_sb = small.tile([32, 1], F32, tag="dfree_sb")
            nc.vector.tensor_add(dfree_sb[:, :], dfree2[:, 0:1], dfree2[:, 1:2])
            rh_pre = small.tile([32, 2], F32, tag="rh_pre")
            nc.gpsimd.memset(rh_pre[:, :], 0.0)
            nc.vector.tensor_sub(rh_pre[:, 0:1], rowmax[:, :], dfree_sb[:, :])
            rowhinge = small.tile([32, 2], F32R, tag="rowhinge")
            nc.scalar.activation(rowhinge[:, :], rh_pre[:, :], ACT.Relu,
                                 bias=margin128[0:32, :])
            # place row hinges at global positions: rhpos_h = post_h.T @ rowhinge
            rhpos = small.tile([128, 2], F32, tag="rhpos")
            for h in range(2):
                rh_ps = pepi.tile([128, 2], F32, tag="epi", name="rh_ps")
                nc.tensor.matmul(rh_ps[:, :], lhsT=postr[h][:, :],
                                 rhs=rowhinge[:, :],
                                 start=True, stop=True)
                nc.scalar.copy(rhpos[:, h:h + 1], rh_ps[:, 0:1])

            # payload -> DRAM -> AllGather
            nc.sync.dma_start(out=pay_d[0:128, :], in_=colpart[:, 0:1])
            nc.sync.dma_start(out=pay_d[128:256, :], in_=colpart[:, 1:2])
            nc.sync.dma_start(out=pay_d[256:384, :], in_=dcol[:, 0:1])
            nc.sync.dma_start(out=pay_d[384:512, :], in_=dcol[:, 1:2])
            nc.sync.dma_start(out=pay_d[512:640, :], in_=rhpos[:, 0:1])
            nc.sync.dma_start(out=pay_d[640:768, :], in_=rhpos[:, 1:2])
            nc.gpsimd.collective_compute(
                "AllGather", ALU.bypass,
                replica_groups=[list(range(NCORES))],
                ins=[pay_d.opt()],
                outs=[ag2_d.opt()],
            )

            # final combine (identical on every core)
            ag_sb = small.tile([NCORES, 768], F32, tag="ag_sb")
            nc.sync.dma_start(
                out=ag_sb[:, :],
                in_=ag2_d.rearrange("(m x) o -> m (x o)", m=NCORES),
            )
            finalvec = small.tile([128, 4], F32R, tag="finalvec")
            agg = small.tile([128, 6], F32, tag="agg")
            for c6 in range(6):
                t_ps = pepi.tile([128, NCORES], F32, tag="epi")
                nc.tensor.transpose(t_ps[:, :], ag_sb[:, 128 * c6:128 * (c6 + 1)],
                                    ident[0:NCORES, 0:NCORES])
                nc.vector.tensor_reduce(
                    out=agg[:, c6:c6 + 1], in_=t_ps[:, :], axis=AX.X,
                    op=(ALU.max if c6 < 2 else ALU.add),
                )
            for h in range(2):
                # colhinge_h = relu(colmax_h - dfull_h + margin)
                ch = small.tile([128, 1], F32, tag="ch")
                nc.vector.tensor_sub(ch[:, :], agg[:, h:h + 1], agg[:, 2 + h:3 + h])
                nc.scalar.activation(finalvec[:, h:h + 1], ch[:, :], ACT.Relu,
                                     bias=margin128[:, :])
                nc.scalar.copy(finalvec[:, 2 + h:3 + h], agg[:, 4 + h:5 + h])
            fin_ps = pepi.tile([1, 4], F32, tag="epi")
            nc.tensor.matmul(fin_ps[:, :], lhsT=onesr[:, :],
                             rhs=finalvec[:, :], start=True, stop=True)
            loss = small.tile([1, 1], F32, tag="loss")
            nc.vector.tensor_reduce(out=loss[:, :], in_=fin_ps[:, :], axis=AX.X,
                                    op=ALU.add)
            nc.sync.dma_start(out=out_e[:, :], in_=loss[:, :])

    nc.finalize()
    return nc


# ---------------------------------------------------------------------------
# host side
# ---------------------------------------------------------------------------

def build_in_maps(im_set, s_seq, im_len, s_len):
    im_set = np.asarray(im_set, dtype=np.float32)
    s_seq = np.asarray(s_seq, dtype=np.float32)
    im_len = np.asarray(im_len, dtype=np.int32)
    s_len = np.asarray(s_len, dtype=np.int32)

    s_rows = np.ascontiguousarray(s_seq[:, 1:1 + LS, :].reshape(CJ, D))
    # st[t, p, k, c] = s_rows[128t + c, 128k + p]
    st = np.ascontiguousarray(
        s_rows.reshape(NT, 128, KC, 128).transpose(0, 3, 2, 1))
    gmat = _gmat_host()
    iota36 = np.broadcast_to(np.arange(LI, dtype=np.float32), (BL, LI)).copy()
    iota30 = np.broadcast_to(np.arange(LS, dtype=np.float32), (128, LS)).copy()
    ident = np.eye(128, dtype=np.float32)

    in_maps = []
    for m in range(NCORES):
        pos0, pos1, post0, post1 = _core_masks(m)
        imr = np.ascontiguousarray(
            im_set[BL * m:BL * (m + 1), 1:, :].reshape(BI, D))
        snr = np.ascontiguousarray(s_rows[SJ * m:SJ * (m + 1)])
        in_maps.append({
            "imr": imr,
            "snr": snr,
            "st": st,
            "imlen": np.ascontiguousarray(im_len[BL * m:BL * (m + 1)]),
            "slen": s_len,
            "iota36": iota36,
            "iota30": iota30,
            "ident": ident,
            "gmat": gmat,
            "pos0": pos0,
            "pos1": pos1,
            "post0": post0,
            "post1": post1,
        })
    return in_maps


_NC_CACHE = None


def kernel(im_set, s_seq, im_len, s_len):
    global _NC_CACHE, LAST_RESULT
    if _NC_CACHE is None:
        _NC_CACHE = build_nc()
    nc = _NC_CACHE
    in_maps = build_in_maps(im_set, s_seq, im_len, s_len)
    res = run_bass_kernel_spmd(nc, in_maps, core_ids=list(range(NCORES)))
    LAST_RESULT = res
    out = np.asarray(res.results[0]["out"], dtype=np.float32).reshape(())
    return out


# revision 16
# speedup vs baseline: 1.2281x; 1.2281x over previous
"""Distributed Trainium2 Bass kernel for AlignmentContrastiveLoss.

Reference computation (B=256, L_im=37, L_s=33, D=1024):
    im  = l2norm(im_set)[:, 1:, :]   masked by im_len-1     [B, 36, D]
    s   = l2norm(s_seq)[:, 1:-2, :]  masked by s_len-3      [B, 30, D]
    align[b,c,i,j] = im[b,i] . s[c,j]   (masked entries -> 0)
    scores[b,c] = sum_j max_i align[b,c,i,j]
    loss = sum_b relu(M + max_{c!=b} scores[b,c] - scores[b,b])
         + sum_c relu(M + max_{b!=c} scores[b,c] - scores[c,c])

Sharding: image batch axis across 8 cores (32 images/core); every core
holds the full sentence set (replicated via its input map).  Each core
computes its 32x256 block of scores via fp32r matmuls (PE), max-over-i
on DVE directly from PSUM, the j-sum via small 0/1 "G" matmuls into two
per-core scoresT accumulators [256 x 32], then per-core partial stats
(col-max / diag / row-hinge) are AllGathered (768 floats) and every core
redundantly computes the final scalar.  s norms are computed sharded and
AllGathered (960 floats each) instead of redundantly per-core.
"""

import os
import sys

import numpy as np

for _p in ("/opt/trn_rl_repo", "/root/.axon_site/_ro/trn_rl_repo"):
    if os.path.isdir(_p) and _p not in sys.path:
        sys.path.append(_p)

import concourse.bass as bass
import concourse.mybir as mybir
import concourse.tile as tile
from concourse import bacc
from concourse.bass_utils import run_bass_kernel_spmd


def _ensure_axon_hooks():
    """Some agent images ship an ``antenv`` without ``axon_hooks``, but
    bass_utils hard-imports it when trace=True.  Provide the registry and,
    when libaxon_pjrt.so is available, the real NTFF profile hook."""
    import types

    try:
        import antenv.axon_hooks  # noqa: F401
        return
    except ImportError:
        pass
    try:
        import antenv
    except ImportError:
        return
    mod = types.ModuleType("antenv.axon_hooks")
    mod._hook = None
    mod.set_axon_ntff_profile_hook = lambda h: setattr(mod, "_hook", h)
    mod.get_axon_ntff_profile_hook = lambda: mod._hook
    sys.modules["antenv.axon_hooks"] = mod
    antenv.axon_hooks = mod
    so_path = "/opt/axon/libaxon_pjrt.so"
    try:
        import trn_agent_boot.trn_boot as _tb
        if os.path.exists(so_path):
            mod._hook = _tb._ntff_profile_via_ctypes(so_path)
    except Exception:
        pass


_ensure_axon_hooks()

F32 = mybir.dt.float32
F32R = mybir.dt.float32r
I32 = mybir.dt.int32
AX = mybir.AxisListType
ALU = mybir.AluOpType
ACT = mybir.ActivationFunctionType

NCORES = 8
B, LI, LS, D = 256, 36, 30, 1024
BL = B // NCORES            # 32 images / core
BI = BL * LI                # 1152 im rows / core
CJ = B * LS                 # 7680 (c,j) rows
NT = CJ // 128              # 60 M-tiles
NRT = BI // 128             # 9 im row-tiles
KC = D // 128               # 8 contraction chunks
SJ = CJ // NCORES           # 960 s rows / core (norm shard)
WROWS = 960                 # rows per 32-sentence window
NCHUNKS = [(0, 432, 12), (432, 432, 12), (864, 288, 8)]  # (off, width, n_images)
MARGIN, EPS, NEG = 0.2, 1e-12, -1.0e9

LAST_RESULT = None  # BassKernelResults of the most recent run (for test harness)


# ---------------------------------------------------------------------------
# compile-time tables
# ---------------------------------------------------------------------------

HALF_T = NT // 2  # 30 M-tiles per 128-sentence half


def _gmat_host():
    """G[p, 128t + cl] = 1 where row (128t+p) belongs to local sentence cl
    of tile t's half; G_t.T @ mx_t sums words j into scoresT[half] rows."""
    g = np.zeros((128, NT * 128), np.float32)
    for t in range(NT):
        h = t // HALF_T
        p = np.arange(128)
        cl = (128 * t + p) // LS - 128 * h
        g[p, 128 * t + cl] = 1.0
    return g


def _core_masks(m):
    pos0 = np.zeros((128, 32), np.float32)
    pos1 = np.zeros((128, 32), np.float32)
    tgt = pos0 if m < 4 else pos1
    b = np.arange(32)
    tgt[32 * (m % 4) + b, b] = 1.0
    return pos0, pos1, np.ascontiguousarray(pos0.T), np.ascontiguousarray(pos1.T)


# ---------------------------------------------------------------------------
# device program
# ---------------------------------------------------------------------------

def build_nc():
    nc = bacc.Bacc(None, target_bir_lowering=False, debug=False, num_devices=NCORES)

    imr_e = nc.declare_dram_parameter("imr", [BI, D], F32, isOutput=False)
    snr_e = nc.declare_dram_parameter("snr", [SJ, D], F32, isOutput=False)
    st_e = nc.declare_dram_parameter("st", [NT, 128, KC, 128], F32R, isOutput=False)
    imlen_e = nc.declare_dram_parameter("imlen", [BL], I32, isOutput=False)
    slen_e = nc.declare_dram_parameter("slen", [B], I32, isOutput=False)
    iota36_e = nc.declare_dram_parameter("iota36", [BL, LI], F32, isOutput=False)
    iota30_e = nc.declare_dram_parameter("iota30", [128, LS], F32, isOutput=False)
    ident_e = nc.declare_dram_parameter("ident", [128, 128], F32, isOutput=False)
    gmat_e = nc.declare_dram_parameter("gmat", [128, NT * 128], F32R, isOutput=False)
    pos0_e = nc.declare_dram_parameter("pos0", [128, 32], F32, isOutput=False)
    pos1_e = nc.declare_dram_parameter("pos1", [128, 32], F32, isOutput=False)
    post0_e = nc.declare_dram_parameter("post0", [32, 128], F32, isOutput=False)
    post1_e = nc.declare_dram_parameter("post1", [32, 128], F32, isOutput=False)
    out_e = nc.declare_dram_parameter("out", [1, 1], F32, isOutput=True)

    with tile.TileContext(nc) as tc:
        from contextlib import ExitStack

        with ExitStack() as ctx:
            dram = ctx.enter_context(tc.tile_pool(name="dram", bufs=1, space="DRAM"))
            const = ctx.enter_context(tc.tile_pool(name="const", bufs=1))
            small = ctx.enter_context(tc.tile_pool(name="small", bufs=1))
            stp = ctx.enter_context(tc.tile_pool(name="stp", bufs=3))
            mxp = ctx.enter_context(tc.tile_pool(name="mxp", bufs=4))
            prep = ctx.enter_context(tc.tile_pool(name="prep", bufs=2))
            # PSUM budget (8 banks): align 6 + S accumulator 1 + epi scratch 1
            pal = ctx.enter_context(tc.tile_pool(name="pal", bufs=6, space="PSUM"))

            # DRAM scratch
            imask_d = dram.tile([BI, 1], F32, tag="imask_d")
            smask_d = dram.tile([CJ, 1], F32, tag="smask_d")
            snorm_d = dram.tile([SJ, 1], F32, tag="snorm_d")
            snormall_d = dram.tile([CJ, 1], F32, tag="snormall_d")
            pay_d = dram.tile([128, 6], F32, tag="pay_d")
            ag2_d = dram.tile([NCORES * 128, 6], F32, tag="ag2_d")

            def epi_psum(shape, name):
                return pal.tile(shape, F32, tag="epi", bufs=1, name=name)

            # ---- early consts needed by prep ----
            ident = const.tile([128, 128], F32, tag="ident")
            nc.sync.dma_start(out=ident[:, :], in_=ident_e[:, :])
            iota36 = const.tile([BL, LI], F32, tag="iota36")
            nc.sync.dma_start(out=iota36[:, :], in_=iota36_e[:, :])
            iota30 = const.tile([128, LS], F32, tag="iota30")
            nc.sync.dma_start(out=iota30[:, :], in_=iota30_e[:, :])

            # ---- masks from lengths ----
            imlen_i = small.tile([BL, 1], I32, tag="imlen_i")
            nc.sync.dma_start(out=imlen_i[:, :], in_=imlen_e[:])
            imlen_f = small.tile([BL, 1], F32, tag="imlen_f")
            nc.vector.tensor_copy(imlen_f[:, :], imlen_i[:, :])
            nc.vector.tensor_scalar_add(imlen_f[:, :], imlen_f[:, :], -1.0)
            mask36 = small.tile([BL, LI], F32, tag="mask36")
            nc.vector.tensor_scalar(
                out=mask36[:, :], in0=iota36[:, :], scalar1=imlen_f[:, :],
                scalar2=None, op0=ALU.is_lt,
            )
            nc.sync.dma_start(
                out=imask_d.rearrange("(b i) o -> b (i o)", b=BL),
                in_=mask36[:, :],
            )

            slen_i = small.tile([128, 2], I32, tag="slen_i")
            nc.sync.dma_start(
                out=slen_i[:, :],
                in_=slen_e.ap().rearrange("(h c) -> c h", h=2),
            )
            slen_f = small.tile([128, 2], F32, tag="slen_f")
            nc.vector.tensor_copy(slen_f[:, :], slen_i[:, :])
            nc.vector.tensor_scalar_add(slen_f[:, :], slen_f[:, :], -3.0)
            for h in range(2):
                mask30 = small.tile([128, LS], F32, tag="mask30")
                nc.vector.tensor_scalar(
                    out=mask30[:, :], in0=iota30[:, :], scalar1=slen_f[:, h:h + 1],
                    scalar2=None, op0=ALU.is_lt,
                )
                nc.sync.dma_start(
                    out=smask_d[3840 * h:3840 * (h + 1), :]
                    .rearrange("(c j) o -> c (j o)", c=128),
                    in_=mask30[:, :],
                )

            # ---- phase 1: sharded s sum-of-squares + AllGather ----
            ssq8 = small.tile([120, 8], F32, tag="ssq8")
            for j in range(8):
                snr_t = prep.tile([120, D], F32, tag="ld")
                nc.sync.dma_start(out=snr_t[:, :], in_=snr_e[120 * j:120 * (j + 1), :])
                sq = prep.tile([120, D], F32, tag="sq")
                nc.scalar.activation(sq[:, :], snr_t[:, :], ACT.Square,
                                     accum_out=ssq8[:, j:j + 1])
            # [120, 8] -> [8, 120] so the DRAM write is contiguous per partition
            ssqT_ps = epi_psum([8, 120], "ssqT_ps")
            nc.tensor.transpose(ssqT_ps[:, :], ssq8[:, :], ident[0:120, 0:120])
            ssqT = small.tile([8, 120], F32, tag="ssqT")
            nc.scalar.copy(ssqT[:, :], ssqT_ps[:, :])
            nc.sync.dma_start(
                out=snorm_d.rearrange("(j p) o -> j (p o)", j=8),
                in_=ssqT[:, :],
            )
            nc.gpsimd.collective_compute(
                "AllGather", ALU.bypass,
                replica_groups=[list(range(NCORES))],
                ins=[snorm_d.opt()],
                outs=[snormall_d.opt()],
            )

            # ---- phase 2: im prep (norm+mask+scale, transpose to imt) ----
            # imask reload via [9,128] contiguous load + PE transpose
            imask9 = small.tile([NRT, 128], F32, tag="imask9")
            nc.sync.dma_start(
                out=imask9[:, :],
                in_=imask_d.rearrange("(rt p) o -> rt (p o)", rt=NRT),
            )
            imaskT_ps = epi_psum([128, NRT], "imaskT_ps")
            nc.tensor.transpose(imaskT_ps[:, :], imask9[:, :], ident[0:NRT, 0:NRT])
            imask_sb = small.tile([128, NRT], F32, tag="imask_sb")
            nc.scalar.copy(imask_sb[:, :], imaskT_ps[:, :])

            imt = const.tile([128, KC * BI], F32R, tag="imt")
            for rt in range(NRT):
                imr_t = prep.tile([128, D], F32, tag="ld")
                nc.sync.dma_start(out=imr_t[:, :], in_=imr_e[128 * rt:128 * (rt + 1), :])
                sq = prep.tile([128, D], F32, tag="sq")
                ssq = small.tile([128, 1], F32, tag="imssq")
                nc.scalar.activation(sq[:, :], imr_t[:, :], ACT.Square,
                                     accum_out=ssq[:, :])
                nrm = small.tile([128, 1], F32, tag="imnrm")
                nc.scalar.activation(nrm[:, :], ssq[:, :], ACT.Sqrt)
                nc.vector.tensor_scalar_max(nrm[:, :], nrm[:, :], EPS)
                rcp = small.tile([128, 1], F32, tag="imrcp")
                nc.vector.reciprocal(rcp[:, :], nrm[:, :])
                nc.vector.tensor_mul(rcp[:, :], rcp[:, :], imask_sb[:, rt:rt + 1])
                ims = prep.tile([128, D], F32, tag="ims")
                nc.scalar.mul(ims[:, :], imr_t[:, :], mul=rcp[:, :])
                for k in range(KC):
                    pst = pal.tile([128, 128], F32, tag="al", name="pst")
                    nc.tensor.transpose(pst[:, :], ims[:, 128 * k:128 * (k + 1)],
                                        ident[:, :])
                    nc.scalar.copy(imt[:, BI * k + 128 * rt:BI * k + 128 * (rt + 1)],
                                   pst[:, :])

            # ---- phase 3: s scale vector [128, NT] ----
            # contiguous [60,128] loads + PE transposes (avoid 4B-strided DMA)
            ssq60 = small.tile([NT, 128], F32, tag="ssq60")
            nc.sync.dma_start(
                out=ssq60[:, :],
                in_=snormall_d.rearrange("(t p) o -> t (p o)", t=NT),
            )
            ssqall_ps = epi_psum([128, NT], "ssqall_ps")
            nc.tensor.transpose(ssqall_ps[:, :], ssq60[:, :], ident[0:NT, 0:NT])
            ssqall = small.tile([128, NT], F32, tag="ssqall")
            nc.scalar.copy(ssqall[:, :], ssqall_ps[:, :])

            smask60 = small.tile([NT, 128], F32, tag="smask60")
            nc.sync.dma_start(
                out=smask60[:, :],
                in_=smask_d.rearrange("(t p) o -> t (p o)", t=NT),
            )
            smask_ps = epi_psum([128, NT], "smask_ps")
            nc.tensor.transpose(smask_ps[:, :], smask60[:, :], ident[0:NT, 0:NT])
            smask_sb = small.tile([128, NT], F32, tag="smask_sb")
            nc.scalar.copy(smask_sb[:, :], smask_ps[:, :])

            snrm = small.tile([128, NT], F32, tag="snrm")
            nc.scalar.activation(snrm[:, :], ssqall[:, :], ACT.Sqrt)
            nc.vector.tensor_scalar_max(snrm[:, :], snrm[:, :], EPS)
            sscale = small.tile([128, NT], F32, tag="sscale")
            nc.vector.reciprocal(sscale[:, :], snrm[:, :])
            nc.vector.tensor_mul(sscale[:, :], sscale[:, :], smask_sb[:, :])

            # ---- late consts (needed by main loop G-matmuls / epilogue) ----
            gmat = const.tile([128, NT * 128], F32R, tag="gmat")
            nc.sync.dma_start(out=gmat[:, :], in_=gmat_e[:, :])
            pos0 = const.tile([128, 32], F32, tag="pos0")
            nc.sync.dma_start(out=pos0[:, :], in_=pos0_e[:, :])
            pos1 = const.tile([128, 32], F32, tag="pos1")
            nc.sync.dma_start(out=pos1[:, :], in_=pos1_e[:, :])
            post0 = const.tile([32, 128], F32, tag="post0")
            nc.sync.dma_start(out=post0[:, :], in_=post0_e[:, :])
            post1 = const.tile([32, 128], F32, tag="post1")
            nc.sync.dma_start(out=post1[:, :], in_=post1_e[:, :])
            ones128 = const.tile([128, 1], F32, tag="ones128")
            nc.gpsimd.memset(ones128[:, :], 1.0)
            margin128 = const.tile([128, 1], F32, tag="margin128")
            nc.gpsimd.memset(margin128[:, :], MARGIN)

            # ---- phase 4: main loop over 60 M-tiles ----
            # S halves share one PSUM bank: [128, 64], cols [0:32] half0, [32:64] half1
            psacc = ctx.enter_context(tc.tile_pool(name="psacc", bufs=1, space="PSUM"))
            s_both = psacc.tile([128, 64], F32, tag="Sboth")
            s_ps = [s_both[:, 0:32], s_both[:, 32:64]]

            def emit_scale_g(t):
                mx, _ = pending[t]
                mx_r = mxp.tile([128, 32], F32R, tag="mx_r", name="mx_r")
                nc.scalar.mul(mx_r[:, :], mx[:, :], mul=sscale[:, t:t + 1])
                nc.tensor.matmul(
                    s_ps[t // HALF_T],
                    lhsT=gmat[:, 128 * t:128 * (t + 1)],
                    rhs=mx_r[:, :],
                    start=(t % HALF_T == 0), stop=(t % HALF_T == HALF_T - 1),
                )

            pending = {}
            for t in range(NT):
                st_t = stp.tile([128, KC * 128], F32R, tag="st")
                nc.sync.dma_start(
                    out=st_t.rearrange("p (k c) -> p k c", k=KC),
                    in_=st_e[t, :, :, :],
                )
                # k-outer: one weight per (t,k) feeds all 3 N-chunks
                ps3 = [pal.tile([128, 432], F32, tag="al", name="ps") for _ in range(3)]
                for k in range(KC):
                    for ci, (noff, nw, nimg) in enumerate(NCHUNKS):
                        nc.tensor.matmul(
                            ps3[ci][:, :nw],
                            lhsT=st_t[:, 128 * k:128 * (k + 1)],
                            rhs=imt[:, BI * k + noff:BI * k + noff + nw],
                            start=(k == 0), stop=(k == KC - 1),
                        )
                mx = mxp.tile([128, 32], F32, tag="mx", name="mx")
                ioff = 0
                for ci, (noff, nw, nimg) in enumerate(NCHUNKS):
                    nc.vector.tensor_reduce(
                        out=mx[:, ioff:ioff + nimg],
                        in_=ps3[ci].rearrange("p (g i) -> p g i", i=LI)[:, :nimg, :],
                        axis=AX.X, op=ALU.max,
                    )
                    ioff += nimg
                pending[t] = (mx, None)
                # defer scale+G by 2 tiles so PE never waits on DVE/ACT/AG
                if t - 2 >= 0:
                    emit_scale_g(t - 2)
            emit_scale_g(NT - 2)
            emit_scale_g(NT - 1)

            # ---- phase 5: per-core stats + AllGather + final ----
            posm = [pos0, pos1]
            payload = small.tile([128, 6], F32, tag="payload")
            snd = [small.tile([128, 32], F32, tag=f"snd{h}", name=f"snd{h}")
                   for h in range(2)]
            trash = small.tile([128, 32], F32, tag="trash")
            for h in range(2):
                # diag extraction: accum_out = sum(S * pos) -> payload col 2+h
                nc.vector.scalar_tensor_tensor(
                    out=trash[:, :], in0=s_ps[h], scalar=1.0, in1=posm[h][:, :],
                    op0=ALU.mult, op1=ALU.mult, accum_out=payload[:, 2 + h:3 + h],
                )
                negm = small.tile([128, 32], F32, tag="negm")
                nc.vector.tensor_scalar_mul(negm[:, :], posm[h][:, :], NEG)
                nc.vector.tensor_add(snd[h][:, :], s_ps[h], negm[:, :])
                nc.vector.tensor_reduce(out=payload[:, h:h + 1], in_=snd[h][:, :],
                                        axis=AX.X, op=ALU.max)
            # f32r twins of matmul operands (ACT rounds on write)
            posr = [small.tile([128, 32], F32R, tag=f"posr{h}", name=f"posr{h}")
                    for h in range(2)]
            nc.scalar.copy(posr[0][:, :], pos0[:, :])
            nc.scalar.copy(posr[1][:, :], pos1[:, :])
            postr = [small.tile([32, 128], F32R, tag=f"postr{h}", name=f"postr{h}")
                     for h in range(2)]
            nc.scalar.copy(postr[0][:, :], post0[:, :])
            nc.scalar.copy(postr[1][:, :], post1[:, :])
            dcolr = small.tile([128, 2], F32R, tag="dcolr")
            nc.scalar.copy(dcolr[:, :], payload[:, 2:4])
            onesr = const.tile([128, 1], F32R, tag="onesr")
            nc.scalar.copy(onesr[:, :], ones128[:, :])

            # row max via PE transpose of snd halves
            rm = small.tile([32, 2], F32, tag="rm")
            for h in range(2):
                stp_ps = epi_psum([32, 128], "stp_ps")
                nc.tensor.transpose(stp_ps[:, :], snd[h][:, :], ident[:, :])
                nc.vector.tensor_reduce(out=rm[:, h:h + 1], in_=stp_ps[:, :],
                                        axis=AX.X, op=ALU.max)
            rowmax = small.tile([32, 1], F32, tag="rowmax")
            nc.vector.tensor_max(rowmax[:, :], rm[:, 0:1], rm[:, 1:2])
            # diag in row order; N=2 (fp32r needs even moving dim), cross terms 0
            dfree_ps = epi_psum([32, 2], "dfree_ps")
            nc.tensor.matmul(dfree_ps[:, :], lhsT=posr[0][:, :],
                             rhs=dcolr[:, :], start=True, stop=False)
            nc.tensor.matmul(dfree_ps[:, :], lhsT=posr[1][:, :],
                             rhs=dcolr[:, :], start=False, stop=True)
            dfree2 = small.tile([32, 2], F32, tag="dfree2")
            nc.scalar.copy(dfree2[:, :], dfree_ps[:, :])
            dfree_sb = small.tile([32, 1], F32, tag="dfree_sb")
            nc.vector.tensor_add(dfree_sb[:, :], dfree2[:, 0:1], dfree2[:, 1:2])
            rh_pre = small.tile([32, 2], F32, tag="rh_pre")
            nc.gpsimd.memset(rh_pre[:, :], 0.0)
            nc.vector.tensor_sub(rh_pre[:, 0:1], rowmax[:, :], dfree_sb[:, :])
            rowhinge = small.tile([32, 2], F32R, tag="rowhinge")
            nc.scalar.activation(rowhinge[:, :], rh_pre[:, :], ACT.Relu,
                                 bias=margin128[0:32, :])
            for h in range(2):
                rh_ps = epi_psum([128, 2], "rh_ps")
                nc.tensor.matmul(rh_ps[:, :], lhsT=postr[h][:, :],
                                 rhs=rowhinge[:, :], start=True, stop=True)
                nc.scalar.copy(payload[:, 4 + h:5 + h], rh_ps[:, 0:1])

            # payload -> DRAM (one DMA, contiguous per partition) -> AllGather
            nc.sync.dma_start(out=pay_d[:, :], in_=payload[:, :])
            nc.gpsimd.collective_compute(
                "AllGather", ALU.bypass,
                replica_groups=[list(range(NCORES))],
                ins=[pay_d.opt()],
                outs=[ag2_d.opt()],
            )

            # final combine (identical on every core)
            ag_sb = small.tile([NCORES, 768], F32, tag="ag_sb")
            nc.sync.dma_start(
                out=ag_sb[:, :],
                in_=ag2_d.rearrange("(m p) c -> m (p c)", m=NCORES),
            )
            agv = ag_sb.rearrange("m (p c) -> m p c", c=6)
            finalvec = small.tile([128, 4], F32R, tag="finalvec")
            agg = small.tile([128, 6], F32, tag="agg")
            for c6 in range(6):
                agt = small.tile([NCORES, 128], F32, tag="agt")
                nc.vector.tensor_copy(agt[:, :], agv[:, :, c6])
                t_ps = epi_psum([128, NCORES], "t_ps")
                nc.tensor.transpose(t_ps[:, :], agt[:, :],
                                    ident[0:NCORES, 0:NCORES])
                nc.vector.tensor_reduce(
                    out=agg[:, c6:c6 + 1], in_=t_ps[:, :], axis=AX.X,
                    op=(ALU.max if c6 < 2 else ALU.add),
                )
            for h in range(2):
                # colhinge_h = relu(colmax_h - dfull_h + margin)
                ch = small.tile([128, 1], F32, tag="ch")
                nc.vector.tensor_sub(ch[:, :], agg[:, h:h + 1], agg[:, 2 + h:3 + h])
                nc.scalar.activation(finalvec[:, h:h + 1], ch[:, :], ACT.Relu,
                                     bias=margin128[:, :])
                nc.scalar.copy(finalvec[:, 2 + h:3 + h], agg[:, 4 + h:5 + h])
            fin_ps = epi_psum([1, 4], "fin_ps")
            nc.tensor.matmul(fin_ps[:, :], lhsT=onesr[:, :],
                             rhs=finalvec[:, :], start=True, stop=True)
            loss = small.tile([1, 1], F32, tag="loss")
            nc.vector.tensor_reduce(out=loss[:, :], in_=fin_ps[:, :], axis=AX.X,
                                    op=ALU.add)
            nc.sync.dma_start(out=out_e[:, :], in_=loss[:, :])

    nc.finalize()
    return nc


# ---------------------------------------------------------------------------
# host side
# ---------------------------------------------------------------------------

def build_in_maps(im_set, s_seq, im_len, s_len):
    im_set = np.asarray(im_set, dtype=np.float32)
    s_seq = np.asarray(s_seq, dtype=np.float32)
    im_len = np.asarray(im_len, dtype=np.int32)
    s_len = np.asarray(s_len, dtype=np.int32)

    s_rows = np.ascontiguousarray(s_seq[:, 1:1 + LS, :].reshape(CJ, D))
    # st[t, p, k, c] = s_rows[128t + c, 128k + p]
    st = np.ascontiguousarray(
        s_rows.reshape(NT, 128, KC, 128).transpose(0, 3, 2, 1))
    gmat = _gmat_host()
    iota36 = np.broadcast_to(np.arange(LI, dtype=np.float32), (BL, LI)).copy()
    iota30 = np.broadcast_to(np.arange(LS, dtype=np.float32), (128, LS)).copy()
    ident = np.eye(128, dtype=np.float32)

    in_maps = []
    for m in range(NCORES):
        pos0, pos1, post0, post1 = _core_masks(m)
        imr = np.ascontiguousarray(
            im_set[BL * m:BL * (m + 1), 1:, :].reshape(BI, D))
        snr = np.ascontiguousarray(s_rows[SJ * m:SJ * (m + 1)])
        in_maps.append({
            "imr": imr,
            "snr": snr,
            "st": st,
            "imlen": np.ascontiguousarray(im_len[BL * m:BL * (m + 1)]),
            "slen": s_len,
            "iota36": iota36,
            "iota30": iota30,
            "ident": ident,
            "gmat": gmat,
            "pos0": pos0,
            "pos1": pos1,
            "post0": post0,
            "post1": post1,
        })
    return in_maps


_NC_CACHE = None


def kernel(im_set, s_seq, im_len, s_len):
    global _NC_CACHE, LAST_RESULT
    if _NC_CACHE is None:
        _NC_CACHE = build_nc()
    nc = _NC_CACHE
    in_maps = build_in_maps(im_set, s_seq, im_len, s_len)
    res = run_bass_kernel_spmd(nc, in_maps, core_ids=list(range(NCORES)))
    LAST_RESULT = res
    out = np.asarray(res.results[0]["out"], dtype=np.float32).reshape(())
    return out


# revision 17
# speedup vs baseline: 1.3475x; 1.0971x over previous
"""Distributed Trainium2 Bass kernel for AlignmentContrastiveLoss.

Reference computation (B=256, L_im=37, L_s=33, D=1024):
    im  = l2norm(im_set)[:, 1:, :]   masked by im_len-1     [B, 36, D]
    s   = l2norm(s_seq)[:, 1:-2, :]  masked by s_len-3      [B, 30, D]
    align[b,c,i,j] = im[b,i] . s[c,j]   (masked entries -> 0)
    scores[b,c] = sum_j max_i align[b,c,i,j]
    loss = sum_b relu(M + max_{c!=b} scores[b,c] - scores[b,b])
         + sum_c relu(M + max_{b!=c} scores[b,c] - scores[c,c])

Sharding: image batch axis across 8 cores (32 images/core); every core
holds the full sentence set (replicated via its input map).  Each core
computes its 32x256 block of scores via fp32r matmuls (PE), max-over-i
on DVE directly from PSUM, the j-sum via small 0/1 "G" matmuls into two
per-core scoresT accumulators [256 x 32], then per-core partial stats
(col-max / diag / row-hinge) are AllGathered (768 floats) and every core
redundantly computes the final scalar.  s norms are computed sharded and
AllGathered (960 floats each) instead of redundantly per-core.
"""

import os
import sys

import numpy as np

for _p in ("/opt/trn_rl_repo", "/root/.axon_site/_ro/trn_rl_repo"):
    if os.path.isdir(_p) and _p not in sys.path:
        sys.path.append(_p)

import concourse.bass as bass
import concourse.mybir as mybir
import concourse.tile as tile
from concourse import bacc
from concourse.bass_utils import run_bass_kernel_spmd


def _ensure_axon_hooks():
    """Some agent images ship an ``antenv`` without ``axon_hooks``, but
    bass_utils hard-imports it when trace=True.  Provide the registry and,
    when libaxon_pjrt.so is available, the real NTFF profile hook."""
    import types

    try:
        import antenv.axon_hooks  # noqa: F401
        return
    except ImportError:
        pass
    try:
        import antenv
    except ImportError:
        return
    mod = types.ModuleType("antenv.axon_hooks")
    mod._hook = None
    mod.set_axon_ntff_profile_hook = lambda h: setattr(mod, "_hook", h)
    mod.get_axon_ntff_profile_hook = lambda: mod._hook
    sys.modules["antenv.axon_hooks"] = mod
    antenv.axon_hooks = mod
    so_path = "/opt/axon/libaxon_pjrt.so"
    try:
        import trn_agent_boot.trn_boot as _tb
        if os.path.exists(so_path):
            mod._hook = _tb._ntff_profile_via_ctypes(so_path)
    except Exception:
        pass


_ensure_axon_hooks()

F32 = mybir.dt.float32
F32R = mybir.dt.float32r
BF16 = mybir.dt.bfloat16
I32 = mybir.dt.int32
AX = mybir.AxisListType
ALU = mybir.AluOpType
ACT = mybir.ActivationFunctionType

NCORES = 8
B, LI, LS, D = 256, 36, 30, 1024
BL = B // NCORES            # 32 images / core
BI = BL * LI                # 1152 im rows / core
CJ = B * LS                 # 7680 (c,j) rows
NT = CJ // 128              # 60 M-tiles
NRT = BI // 128             # 9 im row-tiles
KC = D // 128               # 8 contraction chunks
SJ = CJ // NCORES           # 960 s rows / core (norm shard)
WROWS = 960                 # rows per 32-sentence window
NCHUNKS = [(0, 432, 12), (432, 432, 12), (864, 288, 8)]  # (off, width, n_images)
MARGIN, EPS, NEG = 0.2, 1e-12, -1.0e9

LAST_RESULT = None  # BassKernelResults of the most recent run (for test harness)


# ---------------------------------------------------------------------------
# compile-time tables
# ---------------------------------------------------------------------------

HALF_T = NT // 2  # 30 M-tiles per 128-sentence half


def _gmat_host():
    """G[p, 128t + cl] = 1 where row (128t+p) belongs to local sentence cl
    of tile t's half; G_t.T @ mx_t sums words j into scoresT[half] rows."""
    g = np.zeros((128, NT * 128), np.float32)
    for t in range(NT):
        h = t // HALF_T
        p = np.arange(128)
        cl = (128 * t + p) // LS - 128 * h
        g[p, 128 * t + cl] = 1.0
    return g


def _core_masks(m):
    pos0 = np.zeros((128, 32), np.float32)
    pos1 = np.zeros((128, 32), np.float32)
    tgt = pos0 if m < 4 else pos1
    b = np.arange(32)
    tgt[32 * (m % 4) + b, b] = 1.0
    return pos0, pos1, np.ascontiguousarray(pos0.T), np.ascontiguousarray(pos1.T)


# ---------------------------------------------------------------------------
# device program
# ---------------------------------------------------------------------------

def build_nc():
    nc = bacc.Bacc(None, target_bir_lowering=False, debug=False, num_devices=NCORES)

    imr_e = nc.declare_dram_parameter("imr", [BI, D], F32, isOutput=False)
    snr_e = nc.declare_dram_parameter("snr", [SJ, D], F32, isOutput=False)
    st_e = nc.declare_dram_parameter("st", [NT, 128, KC, 128], F32, isOutput=False)
    imlen_e = nc.declare_dram_parameter("imlen", [BL], I32, isOutput=False)
    slen_e = nc.declare_dram_parameter("slen", [B], I32, isOutput=False)
    iota36_e = nc.declare_dram_parameter("iota36", [BL, LI], F32, isOutput=False)
    iota30_e = nc.declare_dram_parameter("iota30", [128, LS], F32, isOutput=False)
    ident_e = nc.declare_dram_parameter("ident", [128, 128], F32, isOutput=False)
    gmat_e = nc.declare_dram_parameter("gmat", [128, NT * 128], F32R, isOutput=False)
    pos0_e = nc.declare_dram_parameter("pos0", [128, 32], F32, isOutput=False)
    pos1_e = nc.declare_dram_parameter("pos1", [128, 32], F32, isOutput=False)
    post0_e = nc.declare_dram_parameter("post0", [32, 128], F32, isOutput=False)
    post1_e = nc.declare_dram_parameter("post1", [32, 128], F32, isOutput=False)
    out_e = nc.declare_dram_parameter("out", [1, 1], F32, isOutput=True)

    with tile.TileContext(nc) as tc:
        from contextlib import ExitStack

        with ExitStack() as ctx:
            dram = ctx.enter_context(tc.tile_pool(name="dram", bufs=1, space="DRAM"))
            const = ctx.enter_context(tc.tile_pool(name="const", bufs=1))
            small = ctx.enter_context(tc.tile_pool(name="small", bufs=1))
            stp = ctx.enter_context(tc.tile_pool(name="stp", bufs=3))
            mxp = ctx.enter_context(tc.tile_pool(name="mxp", bufs=4))
            prep = ctx.enter_context(tc.tile_pool(name="prep", bufs=2))
            # PSUM budget (8 banks): align 6 + S accumulator 1 + epi scratch 1
            pal = ctx.enter_context(tc.tile_pool(name="pal", bufs=6, space="PSUM"))

            # DRAM scratch
            imask_d = dram.tile([BI, 1], F32, tag="imask_d")
            smask_d = dram.tile([CJ, 1], F32, tag="smask_d")
            snorm_d = dram.tile([SJ, 1], F32, tag="snorm_d")
            snormall_d = dram.tile([CJ, 1], F32, tag="snormall_d")
            pay_d = dram.tile([128, 6], F32, tag="pay_d")
            ag2_d = dram.tile([NCORES * 128, 6], F32, tag="ag2_d")

            def epi_psum(shape, name):
                return pal.tile(shape, F32, tag="epi", bufs=1, name=name)

            # ---- early consts needed by prep ----
            ident = const.tile([128, 128], F32, tag="ident")
            nc.sync.dma_start(out=ident[:, :], in_=ident_e[:, :])
            iota36 = const.tile([BL, LI], F32, tag="iota36")
            nc.sync.dma_start(out=iota36[:, :], in_=iota36_e[:, :])
            iota30 = const.tile([128, LS], F32, tag="iota30")
            nc.sync.dma_start(out=iota30[:, :], in_=iota30_e[:, :])

            # ---- masks from lengths ----
            imlen_i = small.tile([BL, 1], I32, tag="imlen_i")
            nc.sync.dma_start(out=imlen_i[:, :], in_=imlen_e[:])
            imlen_f = small.tile([BL, 1], F32, tag="imlen_f")
            nc.vector.tensor_copy(imlen_f[:, :], imlen_i[:, :])
            nc.vector.tensor_scalar_add(imlen_f[:, :], imlen_f[:, :], -1.0)
            mask36 = small.tile([BL, LI], F32, tag="mask36")
            nc.vector.tensor_scalar(
                out=mask36[:, :], in0=iota36[:, :], scalar1=imlen_f[:, :],
                scalar2=None, op0=ALU.is_lt,
            )
            nc.sync.dma_start(
                out=imask_d.rearrange("(b i) o -> b (i o)", b=BL),
                in_=mask36[:, :],
            )

            slen_i = small.tile([128, 2], I32, tag="slen_i")
            nc.sync.dma_start(
                out=slen_i[:, :],
                in_=slen_e.ap().rearrange("(h c) -> c h", h=2),
            )
            slen_f = small.tile([128, 2], F32, tag="slen_f")
            nc.vector.tensor_copy(slen_f[:, :], slen_i[:, :])
            nc.vector.tensor_scalar_add(slen_f[:, :], slen_f[:, :], -3.0)
            for h in range(2):
                mask30 = small.tile([128, LS], F32, tag="mask30")
                nc.vector.tensor_scalar(
                    out=mask30[:, :], in0=iota30[:, :], scalar1=slen_f[:, h:h + 1],
                    scalar2=None, op0=ALU.is_lt,
                )
                nc.sync.dma_start(
                    out=smask_d[3840 * h:3840 * (h + 1), :]
                    .rearrange("(c j) o -> c (j o)", c=128),
                    in_=mask30[:, :],
                )

            # ---- phase 1: sharded s sum-of-squares + AllGather ----
            ssq8 = small.tile([120, 8], F32, tag="ssq8")
            for j in range(8):
                snr_t = prep.tile([120, D], F32, tag="ld")
                nc.sync.dma_start(out=snr_t[:, :], in_=snr_e[120 * j:120 * (j + 1), :])
                sq = prep.tile([120, D], F32, tag="sq")
                nc.scalar.activation(sq[:, :], snr_t[:, :], ACT.Square,
                                     accum_out=ssq8[:, j:j + 1])
            # [120, 8] -> [8, 120] so the DRAM write is contiguous per partition
            ssqT_ps = epi_psum([8, 120], "ssqT_ps")
            nc.tensor.transpose(ssqT_ps[:, :], ssq8[:, :], ident[0:120, 0:120])
            ssqT = small.tile([8, 120], F32, tag="ssqT")
            nc.scalar.copy(ssqT[:, :], ssqT_ps[:, :])
            nc.sync.dma_start(
                out=snorm_d.rearrange("(j p) o -> j (p o)", j=8),
                in_=ssqT[:, :],
            )
            nc.gpsimd.collective_compute(
                "AllGather", ALU.bypass,
                replica_groups=[list(range(NCORES))],
                ins=[snorm_d.opt()],
                outs=[snormall_d.opt()],
            )

            # ---- phase 2: im prep (norm+mask+scale, transpose to imt) ----
            # imask reload via [9,128] contiguous load + PE transpose
            imask9 = small.tile([NRT, 128], F32, tag="imask9")
            nc.sync.dma_start(
                out=imask9[:, :],
                in_=imask_d.rearrange("(rt p) o -> rt (p o)", rt=NRT),
            )
            imaskT_ps = epi_psum([128, NRT], "imaskT_ps")
            nc.tensor.transpose(imaskT_ps[:, :], imask9[:, :], ident[0:NRT, 0:NRT])
            imask_sb = small.tile([128, NRT], F32, tag="imask_sb")
            nc.scalar.copy(imask_sb[:, :], imaskT_ps[:, :])

            imt = const.tile([128, KC * BI], BF16, tag="imt")
            for rt in range(NRT):
                imr_t = prep.tile([128, D], F32, tag="ld")
                nc.sync.dma_start(out=imr_t[:, :], in_=imr_e[128 * rt:128 * (rt + 1), :])
                sq = prep.tile([128, D], F32, tag="sq")
                ssq = small.tile([128, 1], F32, tag="imssq")
                nc.scalar.activation(sq[:, :], imr_t[:, :], ACT.Square,
                                     accum_out=ssq[:, :])
                nrm = small.tile([128, 1], F32, tag="imnrm")
                nc.scalar.activation(nrm[:, :], ssq[:, :], ACT.Sqrt)
                nc.vector.tensor_scalar_max(nrm[:, :], nrm[:, :], EPS)
                rcp = small.tile([128, 1], F32, tag="imrcp")
                nc.vector.reciprocal(rcp[:, :], nrm[:, :])
                nc.vector.tensor_mul(rcp[:, :], rcp[:, :], imask_sb[:, rt:rt + 1])
                ims = prep.tile([128, D], F32, tag="ims")
                nc.vector.tensor_scalar_mul(ims[:, :], imr_t[:, :], rcp[:, :])
                for k in range(KC):
                    pst = pal.tile([128, 128], F32, tag="al", name="pst")
                    nc.tensor.transpose(pst[:, :], ims[:, 128 * k:128 * (k + 1)],
                                        ident[:, :])
                    nc.vector.tensor_copy(imt[:, BI * k + 128 * rt:BI * k + 128 * (rt + 1)],
                                          pst[:, :])

            # ---- phase 3: s scale vector [128, NT] ----
            # contiguous [60,128] loads + PE transposes (avoid 4B-strided DMA)
            ssq60 = small.tile([NT, 128], F32, tag="ssq60")
            nc.sync.dma_start(
                out=ssq60[:, :],
                in_=snormall_d.rearrange("(t p) o -> t (p o)", t=NT),
            )
            ssqall_ps = epi_psum([128, NT], "ssqall_ps")
            nc.tensor.transpose(ssqall_ps[:, :], ssq60[:, :], ident[0:NT, 0:NT])
            ssqall = small.tile([128, NT], F32, tag="ssqall")
            nc.scalar.copy(ssqall[:, :], ssqall_ps[:, :])

            smask60 = small.tile([NT, 128], F32, tag="smask60")
            nc.sync.dma_start(
                out=smask60[:, :],
                in_=smask_d.rearrange("(t p) o -> t (p o)", t=NT),
            )
            smask_ps = epi_psum([128, NT], "smask_ps")
            nc.tensor.transpose(smask_ps[:, :], smask60[:, :], ident[0:NT, 0:NT])
            smask_sb = small.tile([128, NT], F32, tag="smask_sb")
            nc.scalar.copy(smask_sb[:, :], smask_ps[:, :])

            snrm = small.tile([128, NT], F32, tag="snrm")
            nc.scalar.activation(snrm[:, :], ssqall[:, :], ACT.Sqrt)
            nc.vector.tensor_scalar_max(snrm[:, :], snrm[:, :], EPS)
            sscale = small.tile([128, NT], F32, tag="sscale")
            nc.vector.reciprocal(sscale[:, :], snrm[:, :])
            nc.vector.tensor_mul(sscale[:, :], sscale[:, :], smask_sb[:, :])

            # ---- late consts (needed by main loop G-matmuls / epilogue) ----
            gmat = const.tile([128, NT * 128], F32R, tag="gmat")
            nc.sync.dma_start(out=gmat[:, :], in_=gmat_e[:, :])
            pos0 = const.tile([128, 32], F32, tag="pos0")
            nc.sync.dma_start(out=pos0[:, :], in_=pos0_e[:, :])
            pos1 = const.tile([128, 32], F32, tag="pos1")
            nc.sync.dma_start(out=pos1[:, :], in_=pos1_e[:, :])
            post0 = const.tile([32, 128], F32, tag="post0")
            nc.sync.dma_start(out=post0[:, :], in_=post0_e[:, :])
            post1 = const.tile([32, 128], F32, tag="post1")
            nc.sync.dma_start(out=post1[:, :], in_=post1_e[:, :])
            ones128 = const.tile([128, 1], F32, tag="ones128")
            nc.gpsimd.memset(ones128[:, :], 1.0)
            margin128 = const.tile([128, 1], F32, tag="margin128")
            nc.gpsimd.memset(margin128[:, :], MARGIN)

            # ---- phase 4: main loop over 60 M-tiles ----
            # S halves share one PSUM bank: [128, 64], cols [0:32] half0, [32:64] half1
            psacc = ctx.enter_context(tc.tile_pool(name="psacc", bufs=1, space="PSUM"))
            s_both = psacc.tile([128, 64], F32, tag="Sboth")
            s_ps = [s_both[:, 0:32], s_both[:, 32:64]]

            def emit_scale_g(t):
                mx, _ = pending[t]
                mx_r = mxp.tile([128, 32], F32R, tag="mx_r", name="mx_r")
                nc.scalar.mul(mx_r[:, :], mx[:, :], mul=sscale[:, t:t + 1])
                nc.tensor.matmul(
                    s_ps[t // HALF_T],
                    lhsT=gmat[:, 128 * t:128 * (t + 1)],
                    rhs=mx_r[:, :],
                    start=(t % HALF_T == 0), stop=(t % HALF_T == HALF_T - 1),
                )

            pending = {}
            for t in range(NT):
                st_t = stp.tile([128, KC * 128], F32, tag="st")
                nc.sync.dma_start(
                    out=st_t.rearrange("p (k c) -> p k c", k=KC),
                    in_=st_e[t, :, :, :],
                )
                st_bf = stp.tile([128, KC * 128], BF16, tag="st_bf")
                nc.scalar.copy(st_bf[:, :], st_t[:, :])
                # k-outer: one weight per (t,k) feeds all 3 N-chunks
                ps3 = [pal.tile([128, 432], F32, tag="al", name="ps") for _ in range(3)]
                for k in range(KC):
                    for ci, (noff, nw, nimg) in enumerate(NCHUNKS):
                        nc.tensor.matmul(
                            ps3[ci][:, :nw],
                            lhsT=st_bf[:, 128 * k:128 * (k + 1)],
                            rhs=imt[:, BI * k + noff:BI * k + noff + nw],
                            start=(k == 0), stop=(k == KC - 1),
                        )
                mx = mxp.tile([128, 32], F32, tag="mx", name="mx")
                ioff = 0
                for ci, (noff, nw, nimg) in enumerate(NCHUNKS):
                    nc.vector.tensor_reduce(
                        out=mx[:, ioff:ioff + nimg],
                        in_=ps3[ci].rearrange("p (g i) -> p g i", i=LI)[:, :nimg, :],
                        axis=AX.X, op=ALU.max,
                    )
                    ioff += nimg
                pending[t] = (mx, None)
                # defer scale+G by 2 tiles so PE never waits on DVE/ACT/AG
                if t - 2 >= 0:
                    emit_scale_g(t - 2)
            emit_scale_g(NT - 2)
            emit_scale_g(NT - 1)

            # ---- phase 5: per-core stats + AllGather + final ----
            posm = [pos0, pos1]
            payload = small.tile([128, 6], F32, tag="payload")
            snd = [small.tile([128, 32], F32, tag=f"snd{h}", name=f"snd{h}")
                   for h in range(2)]
            trash = small.tile([128, 32], F32, tag="trash")
            for h in range(2):
                # diag extraction: accum_out = sum(S * pos) -> payload col 2+h
                nc.vector.scalar_tensor_tensor(
                    out=trash[:, :], in0=s_ps[h], scalar=1.0, in1=posm[h][:, :],
                    op0=ALU.mult, op1=ALU.mult, accum_out=payload[:, 2 + h:3 + h],
                )
                negm = small.tile([128, 32], F32, tag="negm")
                nc.vector.tensor_scalar_mul(negm[:, :], posm[h][:, :], NEG)
                nc.vector.tensor_add(snd[h][:, :], s_ps[h], negm[:, :])
                nc.vector.tensor_reduce(out=payload[:, h:h + 1], in_=snd[h][:, :],
                                        axis=AX.X, op=ALU.max)
            # f32r twins of matmul operands (ACT rounds on write)
            posr = [small.tile([128, 32], F32R, tag=f"posr{h}", name=f"posr{h}")
                    for h in range(2)]
            nc.scalar.copy(posr[0][:, :], pos0[:, :])
            nc.scalar.copy(posr[1][:, :], pos1[:, :])
            postr = [small.tile([32, 128], F32R, tag=f"postr{h}", name=f"postr{h}")
                     for h in range(2)]
            nc.scalar.copy(postr[0][:, :], post0[:, :])
            nc.scalar.copy(postr[1][:, :], post1[:, :])
            dcolr = small.tile([128, 2], F32R, tag="dcolr")
            nc.scalar.copy(dcolr[:, :], payload[:, 2:4])
            onesr = const.tile([128, 1], F32R, tag="onesr")
            nc.scalar.copy(onesr[:, :], ones128[:, :])

            # row max via PE transpose of snd halves
            rm = small.tile([32, 2], F32, tag="rm")
            for h in range(2):
                stp_ps = epi_psum([32, 128], "stp_ps")
                nc.tensor.transpose(stp_ps[:, :], snd[h][:, :], ident[:, :])
                nc.vector.tensor_reduce(out=rm[:, h:h + 1], in_=stp_ps[:, :],
                                        axis=AX.X, op=ALU.max)
            rowmax = small.tile([32, 1], F32, tag="rowmax")
            nc.vector.tensor_max(rowmax[:, :], rm[:, 0:1], rm[:, 1:2])
            # diag in row order; N=2 (fp32r needs even moving dim), cross terms 0
            dfree_ps = epi_psum([32, 2], "dfree_ps")
            nc.tensor.matmul(dfree_ps[:, :], lhsT=posr[0][:, :],
                             rhs=dcolr[:, :], start=True, stop=False)
            nc.tensor.matmul(dfree_ps[:, :], lhsT=posr[1][:, :],
                             rhs=dcolr[:, :], start=False, stop=True)
            dfree2 = small.tile([32, 2], F32, tag="dfree2")
            nc.scalar.copy(dfree2[:, :], dfree_ps[:, :])
            dfree_sb = small.tile([32, 1], F32, tag="dfree_sb")
            nc.vector.tensor_add(dfree_sb[:, :], dfree2[:, 0:1], dfree2[:, 1:2])
            rh_pre = small.tile([32, 2], F32, tag="rh_pre")
            nc.gpsimd.memset(rh_pre[:, :], 0.0)
            nc.vector.tensor_sub(rh_pre[:, 0:1], rowmax[:, :], dfree_sb[:, :])
            rowhinge = small.tile([32, 2], F32R, tag="rowhinge")
            nc.scalar.activation(rowhinge[:, :], rh_pre[:, :], ACT.Relu,
                                 bias=margin128[0:32, :])
            for h in range(2):
                rh_ps = epi_psum([128, 2], "rh_ps")
                nc.tensor.matmul(rh_ps[:, :], lhsT=postr[h][:, :],
                                 rhs=rowhinge[:, :], start=True, stop=True)
                nc.scalar.copy(payload[:, 4 + h:5 + h], rh_ps[:, 0:1])

            # payload -> DRAM (one DMA, contiguous per partition) -> AllGather
            nc.sync.dma_start(out=pay_d[:, :], in_=payload[:, :])
            nc.gpsimd.collective_compute(
                "AllGather", ALU.bypass,
                replica_groups=[list(range(NCORES))],
                ins=[pay_d.opt()],
                outs=[ag2_d.opt()],
            )

            # final combine (identical on every core)
            ag_sb = small.tile([NCORES, 768], F32, tag="ag_sb")
            nc.sync.dma_start(
                out=ag_sb[:, :],
                in_=ag2_d.rearrange("(m p) c -> m (p c)", m=NCORES),
            )
            agv = ag_sb.rearrange("m (p c) -> m p c", c=6)
            finalvec = small.tile([128, 4], F32R, tag="finalvec")
            agg = small.tile([128, 6], F32, tag="agg")
            for c6 in range(6):
                agt = small.tile([NCORES, 128], F32, tag="agt")
                nc.vector.tensor_copy(agt[:, :], agv[:, :, c6])
                t_ps = epi_psum([128, NCORES], "t_ps")
                nc.tensor.transpose(t_ps[:, :], agt[:, :],
                                    ident[0:NCORES, 0:NCORES])
                nc.vector.tensor_reduce(
                    out=agg[:, c6:c6 + 1], in_=t_ps[:, :], axis=AX.X,
                    op=(ALU.max if c6 < 2 else ALU.add),
                )
            for h in range(2):
                # colhinge_h = relu(colmax_h - dfull_h + margin)
                ch = small.tile([128, 1], F32, tag="ch")
                nc.vector.tensor_sub(ch[:, :], agg[:, h:h + 1], agg[:, 2 + h:3 + h])
                nc.scalar.activation(finalvec[:, h:h + 1], ch[:, :], ACT.Relu,
                                     bias=margin128[:, :])
                nc.scalar.copy(finalvec[:, 2 + h:3 + h], agg[:, 4 + h:5 + h])
            fin_ps = epi_psum([1, 4], "fin_ps")
            nc.tensor.matmul(fin_ps[:, :], lhsT=onesr[:, :],
                             rhs=finalvec[:, :], start=True, stop=True)
            loss = small.tile([1, 1], F32, tag="loss")
            nc.vector.tensor_reduce(out=loss[:, :], in_=fin_ps[:, :], axis=AX.X,
                                    op=ALU.add)
            nc.sync.dma_start(out=out_e[:, :], in_=loss[:, :])

    nc.finalize()
    return nc


# ---------------------------------------------------------------------------
# host side
# ---------------------------------------------------------------------------

def build_in_maps(im_set, s_seq, im_len, s_len):
    im_set = np.asarray(im_set, dtype=np.float32)
    s_seq = np.asarray(s_seq, dtype=np.float32)
    im_len = np.asarray(im_len, dtype=np.int32)
    s_len = np.asarray(s_len, dtype=np.int32)

    s_rows = np.ascontiguousarray(s_seq[:, 1:1 + LS, :].reshape(CJ, D))
    # st[t, p, k, c] = s_rows[128t + c, 128k + p]
    st = np.ascontiguousarray(
        s_rows.reshape(NT, 128, KC, 128).transpose(0, 3, 2, 1))
    gmat = _gmat_host()
    iota36 = np.broadcast_to(np.arange(LI, dtype=np.float32), (BL, LI)).copy()
    iota30 = np.broadcast_to(np.arange(LS, dtype=np.float32), (128, LS)).copy()
    ident = np.eye(128, dtype=np.float32)

    in_maps = []
    for m in range(NCORES):
        pos0, pos1, post0, post1 = _core_masks(m)
        imr = np.ascontiguousarray(
            im_set[BL * m:BL * (m + 1), 1:, :].reshape(BI, D))
        snr = np.ascontiguousarray(s_rows[SJ * m:SJ * (m + 1)])
        in_maps.append({
            "imr": imr,
            "snr": snr,
            "st": st,
            "imlen": np.ascontiguousarray(im_len[BL * m:BL * (m + 1)]),
            "slen": s_len,
            "iota36": iota36,
            "iota30": iota30,
            "ident": ident,
            "gmat": gmat,
            "pos0": pos0,
            "pos1": pos1,
            "post0": post0,
            "post1": post1,
        })
    return in_maps


_NC_CACHE = None


def kernel(im_set, s_seq, im_len, s_len):
    global _NC_CACHE, LAST_RESULT
    if _NC_CACHE is None:
        _NC_CACHE = build_nc()
    nc = _NC_CACHE
    in_maps = build_in_maps(im_set, s_seq, im_len, s_len)
    res = run_bass_kernel_spmd(nc, in_maps, core_ids=list(range(NCORES)))
    LAST_RESULT = res
    out = np.asarray(res.results[0]["out"], dtype=np.float32).reshape(())
    return out
